# revision 1
# baseline (speedup 1.0000x reference)
"""GroupedQueryAttention Trainium2 kernel (v4).

Reference computation (N=4, L=1024, E=2048, 32 heads of dim 64):
  energy[n,h,q,k] = sum_d Q[n,q,h*64+d] * K[n,k,h*64+d]
  attn = softmax(energy / sqrt(2048), axis=k)
  O[n,q,h*64+d]  = sum_k attn[n,h,q,k] * V[n,k,h*64+d]
  Y = O @ W_out.T + b_out

Sharding (8 cores): data-parallel over N (4) x tensor-parallel over head
halves (2).  Core c handles batch c//2 and heads [16*(c%2), 16*(c%2)+16),
computes its partial fc_out contribution O_half @ W_out[:, cols]^T, and the
host sums the two partials per batch and adds the bias.

The v1 baseline (227 us) was scalar-engine bound: the attention phase is
gated by 128 exp ACTIVATEs (~147 us) while the PE sat at ~70%.  v4:
 - 4 of every 16 k-chunks per head pair compute exp on the vector engine
   instead, via the Schraudolph bf16-bits trick in one tensor_scalar:
   bf16_bits = trunc(S*a + b) approximates exp(S*scale) with a ~3%
   sawtooth; the mixed error measures 8.5e-3 rel_fro (budget 2e-2).
   This rebalances the attention phase to ~120 us across both engines.
 - O'/denominator PSUM evacuation moved off the scalar engine (vector),
   chunk-level software pipelining (S(k+1) and exp(k+1) emitted before
   O'(k)), finer startup and fc-output DMA.
 (Row-tiled 64-row S matmuls were tried and reverted: on this silicon the
  two row-group matmuls serialize AND the half-idle array re-throttles the
  HAM clock gate to 1.2 GHz; zero-padded 128-row contraction keeps K=8/8.)
"""

import sys

sys.path.insert(0, "/opt/trn_rl_repo")

import math

import numpy as np

import ml_dtypes

import concourse.bass as bass
import concourse.mybir as mybir
import concourse.tile as tile
from concourse import bass_utils
from concourse.bass_utils import run_bass_kernel_spmd


N, L, E = 4, 1024, 2048
HEADS, D = 32, 64
HPC = 16          # heads per core
EC = HPC * D      # e-columns per core (1024)
P = 128
SCALE = 1.0 / math.sqrt(float(E))
F32 = mybir.dt.float32
BF16 = mybir.dt.bfloat16
I16 = mybir.dt.int16

# Schraudolph bf16-bits exp: bits = trunc(S*SCH_A + SCH_B); value =
# 2^(S*SCALE*log2e) * (1 + sawtooth(~3%)).  SCH_C centers the sawtooth.
SCH_C = -0.05
SCH_A = 128.0 * math.log2(math.e) * SCALE
SCH_B = 128.0 * (127.0 + SCH_C)

# which k-chunks use the DVE Schraudolph exp, by head parity.
DVE_KC = {0: (4, 6), 1: (4, 6)}


def _dedupe_ldweights(nc):
    """bf16 matmuls are emitted as explicit Ldweights+Matmult pairs, one pair
    per matmul.  Consecutive matmuls sharing the same stationary operand
    reload it needlessly.  Replace a Ldweights whose operand is identical to
    the previous one on the PE stream (with only Matmult/NoOp/EventSemaphore
    in between) by a NoOp that preserves its sync_info."""
    n_drop = 0
    for fn in nc.m.functions:
        stack = list(fn.blocks)
        while stack:
            bb = stack.pop()
            sub = getattr(bb, "blocks", None)
            if sub:
                stack.extend(sub)
            last_key = [None]
            new_insts = []
            for inst in bb.instructions:
                if str(inst.engine) not in ("EngineType.PE", "PE"):
                    new_insts.append(inst)
                    continue
                if inst.opcode == "Ldweights":
                    key = (
                        repr(inst.ins[0]),
                        str(inst.tile_position),
                        str(inst.tile_size),
                    )
                    if key == last_key[0]:
                        nop = mybir.InstNoOp(
                            name=inst.name,
                            engine=inst.engine,
                            ins=[],
                            outs=[],
                            sync_info=inst.sync_info,
                        )
                        new_insts.append(nop)
                        n_drop += 1
                    else:
                        last_key[0] = key
                        new_insts.append(inst)
                elif inst.opcode in ("Matmult", "NoOp", "EventSemaphore"):
                    new_insts.append(inst)
                else:
                    last_key[0] = None
                    new_insts.append(inst)
            bb.instructions = new_insts
    return n_drop


def _split_multi_waits(nc):
    """walrus in this image rejects >1 sem wait per instruction; hoist
    extra waits onto NoOps right before the instruction (same engine)."""
    n_split = 0
    for fn in nc.m.functions:
        stack = list(fn.blocks)
        while stack:
            bb = stack.pop()
            sub = getattr(bb, "blocks", None)
            if sub:
                stack.extend(sub)
            new_insts = []
            for inst in bb.instructions:
                si = inst.sync_info
                if si is not None and len(si.on_wait) > 1:
                    waits = list(si.on_wait)
                    for j, w in enumerate(waits[:-1]):
                        nop = mybir.InstNoOp(
                            name=f"{inst.name}_hw{j}",
                            engine=inst.engine,
                            ins=[],
                            outs=[],
                            sync_info=mybir.SyncInfo(on_wait=[w], on_update=[]),
                        )
                        new_insts.append(nop)
                        n_split += 1
                    si.on_wait = [waits[-1]]
                new_insts.append(inst)
            bb.instructions = new_insts
    return n_split


def _build_program():
    nc = bass.Bass()
    qt = nc.declare_dram_parameter("qt", [HPC * P, L], BF16, isOutput=False)
    kt = nc.declare_dram_parameter("kt", [HPC * P, L], BF16, isOutput=False)
    vh = nc.declare_dram_parameter("vh", [L, HPC * 65], BF16, isOutput=False)
    wt = nc.declare_dram_parameter("wt", [EC, E], BF16, isOutput=False)
    yp = nc.declare_dram_parameter("yp", [L, E], BF16, isOutput=True)

    with tile.TileContext(nc) as tc:
        with tc.tile_pool(name="persist", bufs=1) as persist:
            wt_sb = persist.tile([P, 8, E], BF16)
            ot = persist.tile([P, 8, L], BF16)
            rb_full = persist.tile([P, 8, L], F32)
            den_d = persist.tile([HPC, L], F32, space="DRAM")
            rec_d = persist.tile([HPC, L], F32, space="DRAM")
            with (
                tc.tile_pool(name="io", bufs=3) as io,
                tc.tile_pool(name="apool", bufs=6) as apool,
                tc.tile_pool(name="npool", bufs=3) as npool,
                tc.tile_pool(name="ps_s", bufs=2, space="PSUM") as ps_s,
                tc.tile_pool(name="ps_o", bufs=2, space="PSUM") as ps_o,
            ):
                # HAM warmup: dummy matmuls on a zeroed tile run during the
                # fixed preamble + first DMAs, so the clock gate is at 8/8
                # before the first real matmul.  Two s-tag allocations keep
                # the PSUM rotation parity unchanged.
                wtile = apool.tile([P, 64], BF16, tag="warm", name="wtile")
                nc.vector.memset(wtile[:], 0.0)
                for _wi in range(2):
                    wps = ps_s.tile([P, L], F32, tag="s", name="wps")
                    for _wj in range(14):
                        nc.tensor.matmul(
                            wps[:64, _wj * 64 : (_wj + 1) * 64],
                            wtile[:],
                            wtile[:],
                            start=True,
                            stop=True,
                        )
                for h in range(HPC):
                    hp, hi = h // 2, h % 2
                    po = hi * 64
                    qt2 = io.tile([P, L], BF16, tag="qt2")
                    kt2 = io.tile([P, L], BF16, tag="kt2")
                    vh2 = io.tile([P, 8, 65], BF16, tag="vh2")
                    if h == 0:
                        # fine-grained first-head DMA so the first matmul
                        # starts as early as possible (S chunk 0 only needs
                        # kt cols 0:128 and qt cols 0:512)
                        nc.sync.dma_start(kt2[:, 0:128], kt[0:P, 0:128])
                        nc.sync.dma_start(qt2[:, 0:512], qt[0:P, 0:512])
                        nc.sync.dma_start(qt2[:, 512:1024], qt[0:P, 512:1024])
                        nc.sync.dma_start(kt2[:, 128:512], kt[0:P, 128:512])
                    else:
                        nc.sync.dma_start(kt2[:], kt[h * P : (h + 1) * P, :])
                        nc.sync.dma_start(qt2[:], qt[h * P : (h + 1) * P, :])
                    nc.sync.dma_start(
                        vh2[:],
                        vh[:, h * 65 : (h + 1) * 65].rearrange(
                            "(c p) f -> p c f", p=P
                        ),
                    )
                    if h == 0:
                        nc.sync.dma_start(kt2[:, 512:1024], kt[0:P, 512:1024])
                    if h < 8:  # stage fc weights behind the head inputs
                        nc.sync.dma_start(
                            wt_sb[:, h, :], wt[h * P : (h + 1) * P, :]
                        )
                    o_ps = ps_o.tile([P, L], F32, tag="o")
                    a_tiles = {}
                    # software-pipelined at chunk granularity with lag 2:
                    # O'(kc-2) is emitted after S(kc)+exp(kc), giving every
                    # exp two chunk-periods of latency slack before the
                    # in-order PE queue needs its output.
                    for kc in range(10):
                        if kc < 8:
                            s_ps = ps_s.tile([P, L], F32, tag="s")
                            lhsT = kt2[:, kc * P : (kc + 1) * P]
                            for qc in range(2):
                                nc.tensor.matmul(
                                    s_ps[:, qc * 512 : (qc + 1) * 512],
                                    lhsT,
                                    qt2[:, qc * 512 : (qc + 1) * 512],
                                    start=True,
                                    stop=True,
                                )
                            a_sb = apool.tile([P, L], BF16, tag="a")
                            if kc in DVE_KC[hi]:
                                nc.vector.tensor_scalar(
                                    a_sb[:].bitcast(I16),
                                    s_ps[:],
                                    SCH_A,
                                    SCH_B,
                                    mybir.AluOpType.mult,
                                    mybir.AluOpType.add,
                                )
                            else:
                                nc.scalar.activation(
                                    a_sb[:],
                                    s_ps[:],
                                    mybir.ActivationFunctionType.Exp,
                                    scale=SCALE,
                                )
                            a_tiles[kc] = a_sb
                        if kc >= 2:
                            kp = kc - 2
                            a_sb = a_tiles.pop(kp)
                            vsl = vh2[:, kp, :]
                            for qc in range(2):
                                nc.tensor.matmul(
                                    o_ps[:65, qc * 512 : (qc + 1) * 512],
                                    vsl,
                                    a_sb[:, qc * 512 : (qc + 1) * 512],
                                    start=(kp == 0),
                                    stop=(kp == 7),
                                )
                    # evacuate PSUM: raw (unnormalized) head output and the
                    # denominator row, both on the vector engine.
                    nc.vector.tensor_copy(
                        out=ot[po : po + 64, hp, :], in_=o_ps[:64, :]
                    )
                    den_t = npool.tile([1, L], F32, tag="den")
                    nc.vector.tensor_copy(out=den_t[:], in_=o_ps[64:65, :])
                    nc.sync.dma_start(den_d[h : h + 1, :], den_t[:])
                    if hi == 1:
                        # normalize this finished pair's OT chunk in place,
                        # overlapped with the next heads' attention
                        j = hp
                        dsq = npool.tile([HPC, P], F32, tag="dsq")
                        nc.sync.dma_start(
                            dsq[:],
                            den_d[2 * j : 2 * j + 2, :].rearrange(
                                "h (a b) -> (h a) b", b=P
                            ),
                        )
                        rsq = npool.tile([HPC, P], F32, tag="rsq")
                        nc.vector.reciprocal(rsq[:], dsq[:])
                        nc.sync.dma_start(
                            rec_d[2 * j : 2 * j + 2, :].rearrange(
                                "h (a b) -> (h a) b", b=P
                            ),
                            rsq[:],
                        )
                        for ii in range(2):
                            nc.sync.dma_start(
                                rb_full[ii * 64 : (ii + 1) * 64, j, :],
                                rec_d[
                                    2 * j + ii : 2 * j + ii + 1, :
                                ].to_broadcast((64, L)),
                            )
                        nc.vector.tensor_mul(
                            ot[:, j, :], ot[:, j, :], rb_full[:, j, :]
                        )
                # keep the PE busy across the attention->fc boundary (the
                # pair-7 normalize chain takes ~4us): idle >3.4us would
                # re-throttle the HAM clock gate and start fc cold.
                for _wi in range(2):
                    wps = ps_s.tile([P, L], F32, tag="s", name="wps2")
                    for _wj in range(14):
                        nc.tensor.matmul(
                            wps[:64, _wj * 64 : (_wj + 1) * 64],
                            wtile[:],
                            wtile[:],
                            start=True,
                            stop=True,
                        )

            with (
                tc.tile_pool(name="ysb", bufs=4) as ysbp,
                tc.tile_pool(name="ps_y", bufs=2, space="PSUM") as ps_y,
            ):
                for lc in range(8):
                    y_ps = ps_y.tile([P, E], F32, tag="y")
                    for ec in range(8):
                        lhsT = ot[:, ec, lc * P : (lc + 1) * P]
                        for oc in range(4):
                            nc.tensor.matmul(
                                y_ps[:, oc * 512 : (oc + 1) * 512],
                                lhsT,
                                wt_sb[:, ec, oc * 512 : (oc + 1) * 512],
                                start=(ec == 0),
                                stop=(ec == 7),
                            )
                    nq = 4 if lc == 7 else 2  # last strip drains in quarters
                    w = E // nq
                    for part in range(nq):
                        y_sb = ysbp.tile([P, w], BF16, tag=f"ysb{nq}", name="y_sb")
                        if part % 2 == 0 or lc < 7:
                            nc.scalar.activation(
                                y_sb[:],
                                y_ps[:, part * w : (part + 1) * w],
                                mybir.ActivationFunctionType.Copy,
                            )
                        else:
                            nc.vector.tensor_copy(
                                out=y_sb[:], in_=y_ps[:, part * w : (part + 1) * w]
                            )
                        nc.sync.dma_start(
                            yp[lc * P : (lc + 1) * P, part * w : (part + 1) * w],
                            y_sb[:],
                        )

    _dedupe_ldweights(nc)
    _split_multi_waits(nc)
    return nc


_NC_CACHE = []


def kernel(values, keys, queries, mask, W_out, b_out):
    values = np.asarray(values, dtype=np.float32)
    keys = np.asarray(keys, dtype=np.float32)
    queries = np.asarray(queries, dtype=np.float32)
    W_out = np.asarray(W_out, dtype=np.float32)
    b_out = np.asarray(b_out, dtype=np.float32)

    if not _NC_CACHE:
        _NC_CACHE.append(_build_program())
    nc = _NC_CACHE[0]

    in_maps = []
    for c in range(8):
        n, half = c // 2, c % 2
        cols = slice(half * EC, half * EC + EC)
        qs = queries[n][:, cols].astype(ml_dtypes.bfloat16)
        ks = keys[n][:, cols].astype(ml_dtypes.bfloat16)
        qtm = np.zeros((HPC, P, L), dtype=ml_dtypes.bfloat16)
        ktm = np.zeros((HPC, P, L), dtype=ml_dtypes.bfloat16)
        for hh in range(HPC):
            qtm[hh, :64, :] = qs[:, hh * 64 : (hh + 1) * 64].T
            ktm[hh, :64, :] = ks[:, hh * 64 : (hh + 1) * 64].T
        qtm = qtm.reshape(HPC * P, L)
        ktm = ktm.reshape(HPC * P, L)
        v = values[n][:, cols]
        vhat = np.empty((L, HPC * 65), dtype=ml_dtypes.bfloat16)
        for hh in range(HPC):
            vhat[:, hh * 65 : hh * 65 + 64] = v[:, hh * 64 : (hh + 1) * 64]
            vhat[:, hh * 65 + 64] = 1.0
        wtm = np.ascontiguousarray(W_out[:, cols].T).astype(ml_dtypes.bfloat16)
        in_maps.append({"qt": qtm, "kt": ktm, "vh": vhat, "wt": wtm})

    res = run_bass_kernel_spmd(nc, in_maps, list(range(8)))

    out = np.empty((N, L, E), dtype=np.float32)
    for n in range(N):
        out[n] = (
            res.results[2 * n]["yp"].astype(np.float32)
            + res.results[2 * n + 1]["yp"].astype(np.float32)
            + b_out
        )
    return out

